# revision 40
# baseline (speedup 1.0000x reference)
"""Weighted 2D cross-entropy (BCE-over-classes) loss on 8 Trainium2 cores.

Math (matches the reference):
  t in [0,19); pos = t>0, neg = t==0 (all pixels are pos or neg; mask == 1)
  S(i) = sum_c bce(i,c) = -lnR(i)
     lnR(i) = A(i) + B(i)
     A(i)   = sum_c ln(1-p_c(i))
     B(i)   = ln(p_t(i)) - ln(1-p_t(i)) = ln(e^{-lsel(i)} - 1),  lsel = ln(1-p_t)
  loss = ( (NEG/TOT)*S_pos_sum + (POS/TOT)*S_neg_sum ) / (TOT*C)

Per-core (core k <- batch element k, pure data parallel).  The pixel space
[128, 4096] is processed as two HALVES of [128, 2048]; each unit is one
(class, half): a single 1MB DMA whose per-partition runs are 8KB (vs 4KB
for 0.5MB quarter units -- measured ~26.2 vs ~24.5 GB/s per DMA queue, and
the stream is DMA-bound), one 2048-wide Ln / eq / mask-mult, and 8 matmuls
(512-col) accumulating A and lsel for BOTH quarters of the half into eight
single-bank [128, 512] PSUM half-tiles (all 8 banks).

At the h0->h1 boundary the h1 matmuls must reuse all 8 banks, so the 8
bank-freeing reads are emitted BEFORE the first h1 matmul (Tile tracks only
already-emitted readers -- emitting them later would race): the 4 Exps free
the lsel banks, and 4 DVE copies move the A banks to SBUF so the slow
chained tail (LnB -> STT -> STT) does not gate the PE.  The remaining 12
tail ops then run off the SBUF copies, spread one per unit across h1 (ACT
and DVE are in-order: an op emitted with its producer <1 unit back stalls
the whole engine stream).

Only class 18 of h1 is quarter-split (4 x 0.25MB single-DMA chunks):
(c18,q2) x2 closes q2 before (c18,q3) x2 closes q3, staggering the two
tail chains while keeping the end region -- whose Ln work is tied to the
last-arriving bytes -- as small as possible.

Other schedule facts this build relies on (measured on HW):
  - every unit has a SINGLE DMA, so each Ln needs ONE semaphore wait; a
    2-DMA unit gets its second wait split onto an earlier ACT-queue op
    (1-wait-per-instruction sync structs), stalling the ACT_TABLE_LOAD or
    lock-stepping ACT to the DMA.
  - a 3D [p, c, f] DMA costs ~2.2us of serial descriptor-generation on the
    sync sequencer vs ~0.6us for a plain 2D DMA.
  - tail: expm=Exp(-lsel); B=Ln(expm-1) (fused -1 bias via a [128,1] const
    column); lnR=B+A via STT with accum_out; pos-masked sum via a second
    STT accum.
Target is converted to bf16 on HOST (1MB instead of 2MB int32 DMA, no
on-chip CAST).  Activation tables are pinned to natural_log_exp_and_others
(holds both ln and exp) -- otherwise bacc's table-load pass alternates
between the ln-only and exp-only sets, paying ~1.3us per reload.
Counts (pos/neg) are computed on host from the int target directly.
Per-core output is the raw [128, 16] per-partition stats; the final
partition reduce + 8-way combine happens on host in float64.
"""

from contextlib import ExitStack

import numpy as np

import concourse.bass as bass
import concourse.mybir as mybir
import concourse.tile as tile
from concourse import bacc
from concourse.bass_utils import run_bass_kernel_spmd

# problem shape (hardcoded per harness contract)
N, C, H, W = 8, 19, 512, 1024
PIX = H * W          # 524288 pixels per core
P = 128              # partitions
FCOLS = PIX // P     # 4096 free columns when pixels laid out [128, 4096]
HW2 = FCOLS // 2     # 2048: half width (one 1MB unit)
QW = FCOLS // 4      # 1024: quarter width
HQW = QW // 2        # 512: half-quarter (PSUM bank / matmul / tail width)
N_CORES = 8
NSTAT = 16           # stats columns in the [128, 16] output

DT = mybir.dt

# stats column layout ([128, 16] f32; host folds):
#   2q+h     : sum lnR      for quarter q, half h
#   8+2q+h   : sum pos*lnR  for quarter q, half h
COL_LNR = 0
COL_POSLNR = 8

_ACT_TABLES_PATCHED = False


def _pin_act_table_set():
    """Restrict Ln/Exp to the natural_log_exp_and_others set so bacc's
    table-load pass emits a single ACT_TABLE_LOAD instead of thrashing
    between the ln-only and exp-only sets (~1.3us per reload).  Set
    indices must stay aligned with act_info.json, so every set entry is
    kept -- only the Ln/Exp membership of the other sets is dropped."""
    global _ACT_TABLES_PATCHED
    if _ACT_TABLES_PATCHED:
        return
    import concourse.bacc as bacc_mod

    orig = bacc_mod.get_activation_tables
    ln_exp = {mybir.ActivationFunctionType.Ln, mybir.ActivationFunctionType.Exp}

    def patched(arch):
        tables = orig(arch)
        return {
            name: (fns if name == "natural_log_exp_and_others" else fns - ln_exp)
            for name, fns in tables.items()
        }

    bacc_mod.get_activation_tables = patched
    _ACT_TABLES_PATCHED = True


def build_kernel() -> bass.Bass:
    _pin_act_table_set()

    # Bacc (not raw Bass): its compile() pipeline runs
    # generate_event_semaphores, which splits multi-sem waits to satisfy the
    # 1-wait-per-instruction TRN2 sync structs -- raw Bass modules with
    # Tile-emitted multi-waits fail walrus codegen.
    nc = bacc.Bacc("TRN2")

    predict = nc.declare_dram_parameter("predict", [C, PIX], DT.float32, isOutput=False)
    target = nc.declare_dram_parameter("target", [P, FCOLS], DT.bfloat16, isOutput=False)
    idn = nc.declare_dram_parameter("idn", [P, P], DT.bfloat16, isOutput=False)
    out = nc.declare_dram_parameter("out", [P, NSTAT], DT.float32, isOutput=True)

    pred_r = predict.rearrange("c (p f) -> c p f", p=P)  # [19, 128, 4096]

    with tile.TileContext(nc) as tc, ExitStack() as ctx:
        const = ctx.enter_context(tc.tile_pool(name="const", bufs=1))
        # 1MB half units; bufs=8 aligns slot reuse with the 8 DMA procs and
        # gives 8MB (~19us) of DMA lookahead
        ph_pool = ctx.enter_context(tc.tile_pool(name="ph", bufs=8))
        lmh_pool = ctx.enter_context(tc.tile_pool(name="lmh", bufs=5))
        eqh_pool = ctx.enter_context(tc.tile_pool(name="eqh", bufs=2))
        # 0.25MB chunk-singles for the stream tail (class 18 of h1)
        ps_pool = ctx.enter_context(tc.tile_pool(name="ps", bufs=4))
        lms_pool = ctx.enter_context(tc.tile_pool(name="lms", bufs=2))
        eqs_pool = ctx.enter_context(tc.tile_pool(name="eqs", bufs=2))
        tail_pool = ctx.enter_context(tc.tile_pool(name="tail", bufs=2))
        acp_pool = ctx.enter_context(tc.tile_pool(name="acp", bufs=1))
        psAa_pool = ctx.enter_context(tc.tile_pool(name="psAa", bufs=2, space="PSUM"))
        psAb_pool = ctx.enter_context(tc.tile_pool(name="psAb", bufs=2, space="PSUM"))
        psLa_pool = ctx.enter_context(tc.tile_pool(name="psLa", bufs=2, space="PSUM"))
        psLb_pool = ctx.enter_context(tc.tile_pool(name="psLb", bufs=2, space="PSUM"))

        t_bf = const.tile([P, FCOLS], DT.bfloat16, tag="tb")
        # half 0 of target first so the h0 eq chain is ready before p0
        nc.sync.dma_start(out=t_bf[:, 0:HW2], in_=target[:, 0:HW2])

        idn_sb = const.tile([P, P], DT.bfloat16, tag="idn")
        stats = const.tile([P, NSTAT], DT.float32, tag="stats")
        # per-partition -1.0 bias column for the fused Ln(expm - 1) tail
        negone = const.tile([P, 1], DT.float32, tag="negone")

        state = {"n_dma": 0}

        def count_dma():
            # constants queue behind the first data DMA; the rest of target
            # behind the second -- the h0 pipeline primes first
            state["n_dma"] += 1
            if state["n_dma"] == 1:
                nc.sync.dma_start(out=idn_sb[:], in_=idn[:])
                nc.vector.memset(stats[:], 0.0)
                nc.vector.memset(negone[:], -1.0)
            elif state["n_dma"] == 2:
                nc.sync.dma_start(out=t_bf[:, HW2:], in_=target[:, HW2:])

        def quarter_halves(qsets, qq, s):
            # PSUM half-tile for quarter qq, 512-block s: (l_tile, a_tile)
            return qsets[qq % 2][s]

        def emit_unit(h, c, qsets, start, stop):
            # one (class, half) unit: 1MB single DMA (8KB runs), 2048-wide
            # Ln/eq/mult, 8 matmuls into both quarters' half-banks
            hsl = slice(h * HW2, (h + 1) * HW2)
            p_h = ph_pool.tile([P, HW2], DT.float32, tag="ph")
            nc.sync.dma_start(out=p_h[:, :], in_=pred_r[c, :, hsl])
            count_dma()
            # lm[:, 0, :] = L = Ln(1-p) bf16 ; lm[:, 1, :] = (T==c)*L
            lm = lmh_pool.tile([P, 2, HW2], DT.bfloat16, tag="lmh")
            nc.scalar.activation(
                out=lm[:, 0, :],
                in_=p_h[:, :],
                func=mybir.ActivationFunctionType.Ln,
                bias=1.0,
                scale=-1.0,
            )
            eq = eqh_pool.tile([P, HW2], DT.bfloat16, tag="eqh")
            nc.vector.tensor_scalar(
                out=eq[:, :],
                in0=t_bf[:, hsl],
                scalar1=float(c),
                scalar2=None,
                op0=mybir.AluOpType.is_equal,
            )
            nc.vector.tensor_mul(out=lm[:, 1, :], in0=eq[:, :], in1=lm[:, 0, :])
            # lsel matmuls first: the lsel banks free first in the tail
            for row, which in ((1, "l"), (0, "a")):
                for k in range(4):
                    qq, s = divmod(k, 2)
                    l_t, a_t = qsets[qq][s]
                    dst = l_t if which == "l" else a_t
                    nc.tensor.matmul(
                        dst[:, :],
                        lhsT=idn_sb[:],
                        rhs=lm[:, row, k * HQW : (k + 1) * HQW],
                        start=start,
                        stop=stop,
                    )

        def emit_single(q, c, off, width, qsets, start, stop, src=None):
            # one class's [off, off+width) slice of quarter q: Ln, eq,
            # mask-mult, per-512 matmuls.  src: a preloaded [P, HW2] tile
            # holding the class's whole half (its slice is used directly,
            # no DMA or DMA-wait at the end of the stream).
            qbase = q * QW
            csl = slice(qbase + off, qbase + off + width)
            if src is None:
                p_s = ps_pool.tile([P, QW], DT.float32, tag="ps")
                nc.sync.dma_start(out=p_s[:, :width], in_=pred_r[c, :, csl])
                count_dma()
                p_in = p_s[:, :width]
            else:
                hoff = qbase + off - HW2
                p_in = src[:, hoff : hoff + width]
            lm = lms_pool.tile([P, 2 * QW], DT.bfloat16, tag="lms")
            nc.scalar.activation(
                out=lm[:, :width],
                in_=p_in,
                func=mybir.ActivationFunctionType.Ln,
                bias=1.0,
                scale=-1.0,
            )
            eq = eqs_pool.tile([P, QW], DT.bfloat16, tag="eqs")
            nc.vector.tensor_scalar(
                out=eq[:, :width],
                in0=t_bf[:, csl],
                scalar1=float(c),
                scalar2=None,
                op0=mybir.AluOpType.is_equal,
            )
            nc.vector.tensor_mul(
                out=lm[:, QW : QW + width],
                in0=eq[:, :width],
                in1=lm[:, :width],
            )
            for s in range(width // HQW):
                blk = (off + s * HQW) // HQW
                l_t, a_t = qsets[q % 2][blk]
                nc.tensor.matmul(
                    l_t[:, :],
                    lhsT=idn_sb[:],
                    rhs=lm[:, QW + s * HQW : QW + (s + 1) * HQW],
                    start=start,
                    stop=stop,
                )
            for s in range(width // HQW):
                blk = (off + s * HQW) // HQW
                l_t, a_t = qsets[q % 2][blk]
                nc.tensor.matmul(
                    a_t[:, :],
                    lhsT=idn_sb[:],
                    rhs=lm[:, s * HQW : (s + 1) * HQW],
                    start=start,
                    stop=stop,
                )

        def tail_ops(q, b, l_t, a_src, a_is_psum):
            # tail for 512-block b of quarter q.  a_src is the A source:
            # the PSUM half-tile directly, or an SBUF copy when the bank
            # had to be freed early (h0->h1 boundary).
            # tags per (kind, block) with bufs=2: the two quarters of a half
            # have live tails simultaneously; the next half's allocation then
            # reuses the drained buffers
            col = 2 * q + b
            toff = q * QW + b * HQW
            expm = tail_pool.tile([P, HQW], DT.float32, tag=f"expm{b}")
            bb = tail_pool.tile([P, HQW], DT.float32, tag=f"bb{b}")
            lnr = tail_pool.tile([P, HQW], DT.float32, tag=f"lnr{b}")
            scr = tail_pool.tile([P, HQW], DT.float32, tag=f"scr{b}")

            def op_exp():
                nc.scalar.activation(
                    out=expm[:, :],
                    in_=l_t[:, :],
                    func=mybir.ActivationFunctionType.Exp,
                    scale=-1.0,
                )

            def op_lnb():
                nc.scalar.activation(
                    out=bb[:, :],
                    in_=expm[:, :],
                    func=mybir.ActivationFunctionType.Ln,
                    bias=negone[:],
                )

            def op_lnr():
                nc.vector.scalar_tensor_tensor(
                    out=lnr[:, :],
                    in0=bb[:, :],
                    scalar=0.0,
                    in1=a_src[:, :],
                    op0=mybir.AluOpType.add,
                    op1=mybir.AluOpType.add,
                    accum_out=stats[:, COL_LNR + col : COL_LNR + col + 1],
                )

            def op_scr():
                nc.vector.scalar_tensor_tensor(
                    out=scr[:, :],
                    in0=t_bf[:, toff : toff + HQW],
                    scalar=0.5,
                    in1=lnr[:, :],
                    op0=mybir.AluOpType.is_gt,
                    op1=mybir.AluOpType.mult,
                    accum_out=stats[:, COL_POSLNR + col : COL_POSLNR + col + 1],
                )

            return [op_exp, op_lnb, op_lnr, op_scr]

        def alloc_qsets():
            # 8 single-bank half tiles: qsets[parity][block] = (l, a).
            # One tag per pool: the two per-half allocations rotate through
            # the pool's 2 bufs (parity 0 -> buf0, parity 1 -> buf1), so all
            # four pools together occupy exactly the 8 PSUM banks.
            qs = []
            for par in range(2):
                blocks = []
                for lp, ap, lt, at in (
                    (psLa_pool, psAa_pool, "la", "aa"),
                    (psLb_pool, psAb_pool, "lb", "ab"),
                ):
                    l_t = lp.tile([P, HQW], DT.float32, tag=lt)
                    a_t = ap.tile([P, HQW], DT.float32, tag=at)
                    blocks.append((l_t, a_t))
                qs.append(blocks)
            return qs

        # ---- half 0: quarters 0 (parity 0) and 1 (parity 1) ----
        qsets = alloc_qsets()  # [parity][block] -> (l, a)
        h0_sets = [qsets[0], qsets[1]]  # quarter 0 -> parity 0, quarter 1 -> 1

        for c in range(C):
            emit_unit(0, c, h0_sets, start=(c == 0), stop=(c == C - 1))

        # ---- h0->h1 boundary: free all 8 banks BEFORE any h1 matmul ----
        # Exps free the four lsel banks; DVE copies move the four A banks
        # to SBUF so the chained tail never gates the PE.  These reads MUST
        # be emitted before h1's matmuls: Tile only orders a pool-reuse
        # writer after readers that are already emitted.
        boundary_tails = []  # (remaining ops) spread across h1's units
        for q in (0, 1):
            for b in range(2):
                l_t, a_t = qsets[q % 2][b]
                acp = acp_pool.tile([P, HQW], DT.float32, tag=f"acp{2 * q + b}")
                ops = tail_ops(q, b, l_t, acp, a_is_psum=False)
                ops[0]()  # Exp: frees the lsel bank now
                nc.vector.tensor_copy(out=acp[:, :], in_=a_t[:, :])  # frees A
                boundary_tails.extend(ops[1:])  # LnB, STT, SCR off SBUF copy

        # ---- half 1: quarters 2 (parity 0) and 3 (parity 1) ----
        qsets = alloc_qsets()
        h1_sets = [qsets[0], qsets[1]]

        for c in range(C - 1):
            if c >= 1 and boundary_tails:
                boundary_tails.pop(0)()
            emit_unit(1, c, h1_sets, start=(c == 0), stop=False)
        while boundary_tails:
            boundary_tails.pop(0)()

        # stream tail: only class 18 is quarter-split (4 x 0.25MB chunks) --
        # just enough to close q2 before q3 so the two tail chains stagger,
        # while keeping the end region (and its last-arriving Ln work) small
        emit_single(2, C - 1, 0, HQW, qsets, start=False, stop=True)
        emit_single(2, C - 1, HQW, HQW, qsets, start=False, stop=True)

        t2a = tail_ops(2, 0, qsets[0][0][0], qsets[0][0][1], a_is_psum=True)
        t2b = tail_ops(2, 1, qsets[0][1][0], qsets[0][1][1], a_is_psum=True)
        t2a[0]()  # Exp(q2,a)
        t2b[0]()  # Exp(q2,b)
        emit_single(3, C - 1, 0, HQW, qsets, start=False, stop=True)
        t2a[1]()  # LnB(q2,a)
        t2b[1]()  # LnB(q2,b)
        emit_single(3, C - 1, HQW, HQW, qsets, start=False, stop=True)
        t3a = tail_ops(3, 0, qsets[1][0][0], qsets[1][0][1], a_is_psum=True)
        t3b = tail_ops(3, 1, qsets[1][1][0], qsets[1][1][1], a_is_psum=True)
        t2a[2]()  # STT lnr(q2,a)
        t2b[2]()  # STT lnr(q2,b)
        t3a[0]()  # Exp(q3,a)
        t3b[0]()  # Exp(q3,b)
        t2a[3]()  # SCR(q2,a)
        t2b[3]()  # SCR(q2,b)
        t3a[1]()  # LnB(q3,a)
        t3b[1]()  # LnB(q3,b)
        t3a[2]()
        t3a[3]()
        t3b[2]()
        t3b[3]()

        nc.sync.dma_start(out=out[:], in_=stats[:])

    if not nc.is_finalized():
        nc.finalize()

    return nc


_NC_CACHE = None


def make_in_maps(predict: np.ndarray, target: np.ndarray):
    import ml_dtypes

    predict = np.ascontiguousarray(predict, dtype=np.float32)
    target_bf = np.ascontiguousarray(target, dtype=np.int32).astype(ml_dtypes.bfloat16)
    idn = np.eye(P, dtype=np.float32).astype(ml_dtypes.bfloat16)

    in_maps = []
    for k in range(N_CORES):
        in_maps.append(
            {
                "predict": predict[k].reshape(C, PIX),
                "target": target_bf[k].reshape(P, FCOLS),
                "idn": idn,
            }
        )
    return in_maps


def combine_host(results, target: np.ndarray) -> np.float32:
    tot = np.float64(0.0)
    s_all = np.float64(0.0)
    s_pos = np.float64(0.0)
    for k in range(N_CORES):
        st = results[k]["out"].reshape(P, NSTAT).astype(np.float64)
        s_all += -np.sum(st[:, COL_LNR : COL_LNR + 8])
        s_pos += -np.sum(st[:, COL_POSLNR : COL_POSLNR + 8])
        tot += PIX
    pos = np.float64(np.count_nonzero(target))
    neg = tot - pos
    s_neg = s_all - s_pos
    loss = ((neg / tot) * s_pos + (pos / tot) * s_neg) / (tot * C)
    return np.float32(loss)


def kernel(predict: np.ndarray, target: np.ndarray) -> np.ndarray:
    global _NC_CACHE
    if _NC_CACHE is None:
        _NC_CACHE = build_kernel()
    nc = _NC_CACHE

    in_maps = make_in_maps(predict, target)
    res = run_bass_kernel_spmd(nc, in_maps, list(range(N_CORES)))
    return combine_host(res.results, target)


# revision 41
# speedup vs baseline: 1.1417x; 1.1417x over previous
"""Weighted 2D cross-entropy (BCE-over-classes) loss on 8 Trainium2 cores.

Math (matches the reference):
  t in [0,19); pos = t>0, neg = t==0 (all pixels are pos or neg; mask == 1)
  S(i) = sum_c bce(i,c) = -lnR(i)
     lnR(i) = A(i) + B(i)
     A(i)   = sum_c ln(1-p_c(i))
     B(i)   = ln(p_t(i)) - ln(1-p_t(i)) = ln(e^{-lsel(i)} - 1),  lsel = ln(1-p_t)
  loss = ( (NEG/TOT)*S_pos_sum + (POS/TOT)*S_neg_sum ) / (TOT*C)

Per-core (core k <- batch element k, pure data parallel).  The pixel space
[128, 4096] is processed as two HALVES of [128, 2048]; each unit is one
(class, half): a single 1MB DMA whose per-partition runs are 8KB (vs 4KB
for 0.5MB quarter units -- measured ~26.2 vs ~24.5 GB/s per DMA queue, and
the stream is DMA-bound), one 2048-wide Ln / eq / mask-mult, and 8 matmuls
(512-col) accumulating A and lsel for BOTH quarters of the half into eight
single-bank [128, 512] PSUM half-tiles (all 8 banks).

At the h0->h1 boundary the h1 matmuls must reuse all 8 banks, so the 8
bank-freeing reads are emitted BEFORE the first h1 matmul (Tile tracks only
already-emitted readers -- emitting them later would race): the 4 Exps free
the lsel banks, and 4 DVE copies move the A banks to SBUF so the slow
chained tail (LnB -> STT -> STT) does not gate the PE.  The remaining 12
tail ops then run off the SBUF copies, spread one per unit across h1 (ACT
and DVE are in-order: an op emitted with its producer <1 unit back stalls
the whole engine stream).

Only class 18 of h1 is quarter-split (4 x 0.25MB single-DMA chunks):
(c18,q2) x2 closes q2 before (c18,q3) x2 closes q3, staggering the two
tail chains while keeping the end region -- whose Ln work is tied to the
last-arriving bytes -- as small as possible.

Other schedule facts this build relies on (measured on HW):
  - every unit has a SINGLE DMA, so each Ln needs ONE semaphore wait; a
    2-DMA unit gets its second wait split onto an earlier ACT-queue op
    (1-wait-per-instruction sync structs), stalling the ACT_TABLE_LOAD or
    lock-stepping ACT to the DMA.
  - a 3D [p, c, f] DMA costs ~2.2us of serial descriptor-generation on the
    sync sequencer vs ~0.6us for a plain 2D DMA.
  - tail: expm=Exp(-lsel); B=Ln(expm-1) (fused -1 bias via a [128,1] const
    column); lnR=B+A via STT with accum_out; pos-masked sum via a second
    STT accum.
Target is converted to bf16 on HOST (1MB instead of 2MB int32 DMA, no
on-chip CAST).  Activation tables are pinned to natural_log_exp_and_others
(holds both ln and exp) -- otherwise bacc's table-load pass alternates
between the ln-only and exp-only sets, paying ~1.3us per reload.
Counts (pos/neg) are computed on host from the int target directly.
Per-core output is the raw [128, 16] per-partition stats; the final
partition reduce + 8-way combine happens on host in float64.
"""

from contextlib import ExitStack

import numpy as np

import concourse.bass as bass
import concourse.mybir as mybir
import concourse.tile as tile
from concourse import bacc
from concourse.bass_utils import run_bass_kernel_spmd

# problem shape (hardcoded per harness contract)
N, C, H, W = 8, 19, 512, 1024
PIX = H * W          # 524288 pixels per core
P = 128              # partitions
FCOLS = PIX // P     # 4096 free columns when pixels laid out [128, 4096]
HW2 = FCOLS // 2     # 2048: half width (one 1MB unit)
QW = FCOLS // 4      # 1024: quarter width
HQW = QW // 2        # 512: half-quarter (PSUM bank / matmul / tail width)
N_CORES = 8
NSTAT = 16           # stats columns in the [128, 16] output

DT = mybir.dt

# stats column layout ([128, 16] f32; host folds):
#   2q+h     : sum lnR      for quarter q, half h
#   8+2q+h   : sum pos*lnR  for quarter q, half h
COL_LNR = 0
COL_POSLNR = 8

_ACT_TABLES_PATCHED = False


def _pin_act_table_set():
    """Restrict Ln/Exp to the natural_log_exp_and_others set so bacc's
    table-load pass emits a single ACT_TABLE_LOAD instead of thrashing
    between the ln-only and exp-only sets (~1.3us per reload).  Set
    indices must stay aligned with act_info.json, so every set entry is
    kept -- only the Ln/Exp membership of the other sets is dropped."""
    global _ACT_TABLES_PATCHED
    if _ACT_TABLES_PATCHED:
        return
    import concourse.bacc as bacc_mod

    orig = bacc_mod.get_activation_tables
    ln_exp = {mybir.ActivationFunctionType.Ln, mybir.ActivationFunctionType.Exp}

    def patched(arch):
        tables = orig(arch)
        return {
            name: (fns if name == "natural_log_exp_and_others" else fns - ln_exp)
            for name, fns in tables.items()
        }

    bacc_mod.get_activation_tables = patched
    _ACT_TABLES_PATCHED = True


def build_kernel() -> bass.Bass:
    _pin_act_table_set()

    # Bacc (not raw Bass): its compile() pipeline runs
    # generate_event_semaphores, which splits multi-sem waits to satisfy the
    # 1-wait-per-instruction TRN2 sync structs -- raw Bass modules with
    # Tile-emitted multi-waits fail walrus codegen.
    nc = bacc.Bacc("TRN2")

    predict = nc.declare_dram_parameter("predict", [C, PIX], DT.float32, isOutput=False)
    target = nc.declare_dram_parameter("target", [P, FCOLS], DT.bfloat16, isOutput=False)
    idn = nc.declare_dram_parameter("idn", [P, P], DT.bfloat16, isOutput=False)
    out = nc.declare_dram_parameter("out", [P, NSTAT], DT.float32, isOutput=True)

    pred_r = predict.rearrange("c (p f) -> c p f", p=P)  # [19, 128, 4096]

    with tile.TileContext(nc) as tc, ExitStack() as ctx:
        const = ctx.enter_context(tc.tile_pool(name="const", bufs=1))
        # 1MB half units; bufs=8 aligns slot reuse with the 8 DMA procs and
        # gives 8MB (~19us) of DMA lookahead
        ph_pool = ctx.enter_context(tc.tile_pool(name="ph", bufs=8))
        # bufs=6: one extra unit of Ln->matmul decoupling so the h0->h1
        # boundary PE stall does not back-pressure the ACT stream
        lmh_pool = ctx.enter_context(tc.tile_pool(name="lmh", bufs=6))
        eqh_pool = ctx.enter_context(tc.tile_pool(name="eqh", bufs=2))
        # 0.25MB chunk-singles for the stream tail (class 18 of h1)
        ps_pool = ctx.enter_context(tc.tile_pool(name="ps", bufs=4))
        lms_pool = ctx.enter_context(tc.tile_pool(name="lms", bufs=2))
        eqs_pool = ctx.enter_context(tc.tile_pool(name="eqs", bufs=2))
        tail_pool = ctx.enter_context(tc.tile_pool(name="tail", bufs=2))
        acp_pool = ctx.enter_context(tc.tile_pool(name="acp", bufs=1))
        psAa_pool = ctx.enter_context(tc.tile_pool(name="psAa", bufs=2, space="PSUM"))
        psAb_pool = ctx.enter_context(tc.tile_pool(name="psAb", bufs=2, space="PSUM"))
        psLa_pool = ctx.enter_context(tc.tile_pool(name="psLa", bufs=2, space="PSUM"))
        psLb_pool = ctx.enter_context(tc.tile_pool(name="psLb", bufs=2, space="PSUM"))

        t_bf = const.tile([P, FCOLS], DT.bfloat16, tag="tb")
        # half 0 of target first so the h0 eq chain is ready before p0
        nc.sync.dma_start(out=t_bf[:, 0:HW2], in_=target[:, 0:HW2])

        idn_sb = const.tile([P, P], DT.bfloat16, tag="idn")
        stats = const.tile([P, NSTAT], DT.float32, tag="stats")
        # per-partition -1.0 bias column for the fused Ln(expm - 1) tail
        negone = const.tile([P, 1], DT.float32, tag="negone")

        state = {"n_dma": 0}

        def count_dma():
            # constants queue behind the first data DMA; the rest of target
            # behind the second -- the h0 pipeline primes first
            state["n_dma"] += 1
            if state["n_dma"] == 1:
                nc.sync.dma_start(out=idn_sb[:], in_=idn[:])
                nc.vector.memset(stats[:], 0.0)
                nc.vector.memset(negone[:], -1.0)
            elif state["n_dma"] == 2:
                nc.sync.dma_start(out=t_bf[:, HW2:], in_=target[:, HW2:])

        def quarter_halves(qsets, qq, s):
            # PSUM half-tile for quarter qq, 512-block s: (l_tile, a_tile)
            return qsets[qq % 2][s]

        def emit_unit(h, c, qsets, start, stop):
            # one (class, half) unit: 1MB single DMA (8KB runs), 2048-wide
            # Ln/eq/mult, 8 matmuls into both quarters' half-banks
            hsl = slice(h * HW2, (h + 1) * HW2)
            p_h = ph_pool.tile([P, HW2], DT.float32, tag="ph")
            nc.sync.dma_start(out=p_h[:, :], in_=pred_r[c, :, hsl])
            count_dma()
            # lm[:, 0, :] = L = Ln(1-p) bf16 ; lm[:, 1, :] = (T==c)*L
            lm = lmh_pool.tile([P, 2, HW2], DT.bfloat16, tag="lmh")
            nc.scalar.activation(
                out=lm[:, 0, :],
                in_=p_h[:, :],
                func=mybir.ActivationFunctionType.Ln,
                bias=1.0,
                scale=-1.0,
            )
            eq = eqh_pool.tile([P, HW2], DT.bfloat16, tag="eqh")
            nc.vector.tensor_scalar(
                out=eq[:, :],
                in0=t_bf[:, hsl],
                scalar1=float(c),
                scalar2=None,
                op0=mybir.AluOpType.is_equal,
            )
            nc.vector.tensor_mul(out=lm[:, 1, :], in0=eq[:, :], in1=lm[:, 0, :])
            # lsel matmuls first: the lsel banks free first in the tail
            for row, which in ((1, "l"), (0, "a")):
                for k in range(4):
                    qq, s = divmod(k, 2)
                    l_t, a_t = qsets[qq][s]
                    dst = l_t if which == "l" else a_t
                    nc.tensor.matmul(
                        dst[:, :],
                        lhsT=idn_sb[:],
                        rhs=lm[:, row, k * HQW : (k + 1) * HQW],
                        start=start,
                        stop=stop,
                    )

        def emit_single(q, c, off, width, qsets, start, stop, src=None):
            # one class's [off, off+width) slice of quarter q: Ln, eq,
            # mask-mult, per-512 matmuls.  src: a preloaded [P, HW2] tile
            # holding the class's whole half (its slice is used directly,
            # no DMA or DMA-wait at the end of the stream).
            qbase = q * QW
            csl = slice(qbase + off, qbase + off + width)
            if src is None:
                p_s = ps_pool.tile([P, QW], DT.float32, tag="ps")
                nc.sync.dma_start(out=p_s[:, :width], in_=pred_r[c, :, csl])
                count_dma()
                p_in = p_s[:, :width]
            else:
                hoff = qbase + off - HW2
                p_in = src[:, hoff : hoff + width]
            lm = lms_pool.tile([P, 2 * QW], DT.bfloat16, tag="lms")
            nc.scalar.activation(
                out=lm[:, :width],
                in_=p_in,
                func=mybir.ActivationFunctionType.Ln,
                bias=1.0,
                scale=-1.0,
            )
            eq = eqs_pool.tile([P, QW], DT.bfloat16, tag="eqs")
            nc.vector.tensor_scalar(
                out=eq[:, :width],
                in0=t_bf[:, csl],
                scalar1=float(c),
                scalar2=None,
                op0=mybir.AluOpType.is_equal,
            )
            nc.vector.tensor_mul(
                out=lm[:, QW : QW + width],
                in0=eq[:, :width],
                in1=lm[:, :width],
            )
            for s in range(width // HQW):
                blk = (off + s * HQW) // HQW
                l_t, a_t = qsets[q % 2][blk]
                nc.tensor.matmul(
                    l_t[:, :],
                    lhsT=idn_sb[:],
                    rhs=lm[:, QW + s * HQW : QW + (s + 1) * HQW],
                    start=start,
                    stop=stop,
                )
            for s in range(width // HQW):
                blk = (off + s * HQW) // HQW
                l_t, a_t = qsets[q % 2][blk]
                nc.tensor.matmul(
                    a_t[:, :],
                    lhsT=idn_sb[:],
                    rhs=lm[:, s * HQW : (s + 1) * HQW],
                    start=start,
                    stop=stop,
                )

        def tail_ops(q, b, l_t, a_src, a_is_psum):
            # tail for 512-block b of quarter q.  a_src is the A source:
            # the PSUM half-tile directly, or an SBUF copy when the bank
            # had to be freed early (h0->h1 boundary).
            # tags per (kind, block) with bufs=2: the two quarters of a half
            # have live tails simultaneously; the next half's allocation then
            # reuses the drained buffers
            col = 2 * q + b
            toff = q * QW + b * HQW
            expm = tail_pool.tile([P, HQW], DT.float32, tag=f"expm{b}")
            bb = tail_pool.tile([P, HQW], DT.float32, tag=f"bb{b}")
            lnr = tail_pool.tile([P, HQW], DT.float32, tag=f"lnr{b}")
            scr = tail_pool.tile([P, HQW], DT.float32, tag=f"scr{b}")

            def op_exp():
                nc.scalar.activation(
                    out=expm[:, :],
                    in_=l_t[:, :],
                    func=mybir.ActivationFunctionType.Exp,
                    scale=-1.0,
                )

            def op_lnb():
                nc.scalar.activation(
                    out=bb[:, :],
                    in_=expm[:, :],
                    func=mybir.ActivationFunctionType.Ln,
                    bias=negone[:],
                )

            def op_lnr():
                nc.vector.scalar_tensor_tensor(
                    out=lnr[:, :],
                    in0=bb[:, :],
                    scalar=0.0,
                    in1=a_src[:, :],
                    op0=mybir.AluOpType.add,
                    op1=mybir.AluOpType.add,
                    accum_out=stats[:, COL_LNR + col : COL_LNR + col + 1],
                )

            def op_scr():
                nc.vector.scalar_tensor_tensor(
                    out=scr[:, :],
                    in0=t_bf[:, toff : toff + HQW],
                    scalar=0.5,
                    in1=lnr[:, :],
                    op0=mybir.AluOpType.is_gt,
                    op1=mybir.AluOpType.mult,
                    accum_out=stats[:, COL_POSLNR + col : COL_POSLNR + col + 1],
                )

            return [op_exp, op_lnb, op_lnr, op_scr]

        def alloc_qsets():
            # 8 single-bank half tiles: qsets[parity][block] = (l, a).
            # One tag per pool: the two per-half allocations rotate through
            # the pool's 2 bufs (parity 0 -> buf0, parity 1 -> buf1), so all
            # four pools together occupy exactly the 8 PSUM banks.
            qs = []
            for par in range(2):
                blocks = []
                for lp, ap, lt, at in (
                    (psLa_pool, psAa_pool, "la", "aa"),
                    (psLb_pool, psAb_pool, "lb", "ab"),
                ):
                    l_t = lp.tile([P, HQW], DT.float32, tag=lt)
                    a_t = ap.tile([P, HQW], DT.float32, tag=at)
                    blocks.append((l_t, a_t))
                qs.append(blocks)
            return qs

        # ---- half 0: quarters 0 (parity 0) and 1 (parity 1) ----
        qsets = alloc_qsets()  # [parity][block] -> (l, a)
        h0_sets = [qsets[0], qsets[1]]  # quarter 0 -> parity 0, quarter 1 -> 1

        for c in range(C):
            emit_unit(0, c, h0_sets, start=(c == 0), stop=(c == C - 1))

        # ---- h0->h1 boundary: free all 8 banks BEFORE any h1 matmul ----
        # Exps free the four lsel banks; DVE copies move the four A banks
        # to SBUF so the chained tail never gates the PE.  These reads MUST
        # be emitted before h1's matmuls: Tile only orders a pool-reuse
        # writer after readers that are already emitted.
        boundary_tails = []  # (remaining ops) spread across h1's units
        for q in (0, 1):
            for b in range(2):
                l_t, a_t = qsets[q % 2][b]
                acp = acp_pool.tile([P, HQW], DT.float32, tag=f"acp{2 * q + b}")
                ops = tail_ops(q, b, l_t, acp, a_is_psum=False)
                ops[0]()  # Exp: frees the lsel bank now
                nc.vector.tensor_copy(out=acp[:, :], in_=a_t[:, :])  # frees A
                boundary_tails.extend(ops[1:])  # LnB, STT, SCR off SBUF copy

        # ---- half 1: quarters 2 (parity 0) and 3 (parity 1) ----
        qsets = alloc_qsets()
        h1_sets = [qsets[0], qsets[1]]

        for c in range(C - 1):
            if c >= 1 and boundary_tails:
                boundary_tails.pop(0)()
            emit_unit(1, c, h1_sets, start=(c == 0), stop=False)
        while boundary_tails:
            boundary_tails.pop(0)()

        # stream tail: only class 18 is quarter-split (4 x 0.25MB chunks) --
        # just enough to close q2 before q3 so the two tail chains stagger,
        # while keeping the end region (and its last-arriving Ln work) small
        emit_single(2, C - 1, 0, HQW, qsets, start=False, stop=True)
        emit_single(2, C - 1, HQW, HQW, qsets, start=False, stop=True)

        t2a = tail_ops(2, 0, qsets[0][0][0], qsets[0][0][1], a_is_psum=True)
        t2b = tail_ops(2, 1, qsets[0][1][0], qsets[0][1][1], a_is_psum=True)
        t2a[0]()  # Exp(q2,a)
        t2b[0]()  # Exp(q2,b)
        emit_single(3, C - 1, 0, HQW, qsets, start=False, stop=True)
        t2a[1]()  # LnB(q2,a)
        t2b[1]()  # LnB(q2,b)
        emit_single(3, C - 1, HQW, HQW, qsets, start=False, stop=True)
        t3a = tail_ops(3, 0, qsets[1][0][0], qsets[1][0][1], a_is_psum=True)
        t3b = tail_ops(3, 1, qsets[1][1][0], qsets[1][1][1], a_is_psum=True)
        t2a[2]()  # STT lnr(q2,a)
        t2b[2]()  # STT lnr(q2,b)
        t3a[0]()  # Exp(q3,a)
        t3b[0]()  # Exp(q3,b)
        t2a[3]()  # SCR(q2,a)
        t2b[3]()  # SCR(q2,b)
        t3a[1]()  # LnB(q3,a)
        t3b[1]()  # LnB(q3,b)
        t3a[2]()
        t3a[3]()
        t3b[2]()
        t3b[3]()

        nc.sync.dma_start(out=out[:], in_=stats[:])

    if not nc.is_finalized():
        nc.finalize()

    return nc


_NC_CACHE = None


def make_in_maps(predict: np.ndarray, target: np.ndarray):
    import ml_dtypes

    predict = np.ascontiguousarray(predict, dtype=np.float32)
    target_bf = np.ascontiguousarray(target, dtype=np.int32).astype(ml_dtypes.bfloat16)
    idn = np.eye(P, dtype=np.float32).astype(ml_dtypes.bfloat16)

    in_maps = []
    for k in range(N_CORES):
        in_maps.append(
            {
                "predict": predict[k].reshape(C, PIX),
                "target": target_bf[k].reshape(P, FCOLS),
                "idn": idn,
            }
        )
    return in_maps


def combine_host(results, target: np.ndarray) -> np.float32:
    tot = np.float64(0.0)
    s_all = np.float64(0.0)
    s_pos = np.float64(0.0)
    for k in range(N_CORES):
        st = results[k]["out"].reshape(P, NSTAT).astype(np.float64)
        s_all += -np.sum(st[:, COL_LNR : COL_LNR + 8])
        s_pos += -np.sum(st[:, COL_POSLNR : COL_POSLNR + 8])
        tot += PIX
    pos = np.float64(np.count_nonzero(target))
    neg = tot - pos
    s_neg = s_all - s_pos
    loss = ((neg / tot) * s_pos + (pos / tot) * s_neg) / (tot * C)
    return np.float32(loss)


def kernel(predict: np.ndarray, target: np.ndarray) -> np.ndarray:
    global _NC_CACHE
    if _NC_CACHE is None:
        _NC_CACHE = build_kernel()
    nc = _NC_CACHE

    in_maps = make_in_maps(predict, target)
    res = run_bass_kernel_spmd(nc, in_maps, list(range(N_CORES)))
    return combine_host(res.results, target)
